# revision 33
# baseline (speedup 1.0000x reference)
"""LocalAutoCorr2D Trainium2 kernel.

out[b,c,i,j,dy,dx] = sum_{y,x valid} x[b,c,4i+y,4j+x] * x[b,c,4i+y+sy,4j+x+sx]
with (sy,sx) = (dy-4, dx-4), windows 8x8 at stride 4 on a 96x96 image,
zero-padded at window boundaries.

Strategy (per core, batch-sharded over 8 cores):
  - out[s] == out[-s] (autocorr symmetry) -> only 40 canonical shift classes.
  - x is host-prepped into a PHASE-MAJOR fp16 layout [h, (r, j, c)] with
    w = 4j + r and c innermost, so every matmul rhs view (fixed r, a
    23-j window, all c) is one FLAT contiguous slice: the PE streams at
    full rate (a strided or multi-dim rhs runs at ~half rate). The 5
    vertical shifts v=0..4 are also host-stacked along the free dim, so
    DVE products never need cross-partition operands.
  - Per shift: product Q = x .* shift(x) on the Vector engine (fp16 2x
    mode, flat contiguous views; all shift offsets are multiples of C=64
    elements, so alignment is automatic). Vertical box-sum via 0/1-weight
    matmul (h on partitions), horizontal box-sum folded into PSUM
    accumulation across <=8 matmuls over flat rhs slices of Q.
  - Warmup matmuls bridge the PE from program start until the first
    product is ready: a PE idle gap after ramping can drop the engine
    into a sticky half-clock state for a long stretch.
  - The (0,0) square runs on the Act engine (frees the DVE, starts as
    the first input DMA chunk lands). Scalar evacuates PSUM -> SBUF as
    fp16; each canonical shift is written to DRAM once and the host
    mirrors the 24 symmetric duplicate cells.
"""

import functools
import os
import sys

import numpy as np

sys.path.insert(0, "/opt/trn_rl_repo")

import concourse.bass as bass  # noqa: E402
import concourse.bacc as bacc  # noqa: E402
import concourse.mybir as mybir  # noqa: E402
from concourse import bass_utils  # noqa: E402
from concourse.tile import TileContext  # noqa: E402

B, C, H, W = 8, 64, 96, 96
KH = KW = 8
SH = SW = 4
NH = NW = 23
NCORES = 8

JP = 24           # j' positions per r-block (w = 4j + r)
BLK = C * JP      # 1536 elements per r-block
FLAT = 4 * BLK    # 6144
NV = 5            # vertical shift copies v=0..4 stacked in the free dim
BASE = 64         # leading pad elements (AP validity for negative offsets)
TAIL = 128
XCOLS = BASE + NV * FLAT + TAIL
N_CHUNKS = [(0, 512), (512, 1024), (1024, 1472)]  # flat cols per PSUM bank
N_WARM = 42       # PE warmup: must bridge until the first product is ready

fp32 = mybir.dt.float32
fp16 = mybir.dt.float16


def _canonical_cells():
    """Map canonical shift (sy>=0, sx) -> list of output cells (dy,dx)."""
    cells = {}
    for dy in range(8):
        for dx in range(8):
            sy, sx = dy - 4, dx - 4
            key = (sy, sx) if (sy > 0 or (sy == 0 and sx >= 0)) else (-sy, -sx)
            cells.setdefault(key, []).append((dy, dx))
    assert len(cells) == 40
    return cells


def _amat_np():
    """Vertical box-sum matrices, stacked: A[h, sy*23+i] = 1 if 0<=h-4i<8-sy,
    plus a trailing 23x23 identity block (stage-2 of the T4 reduction)."""
    a = np.zeros((H, 6 * NH), np.float16)
    for sy in range(5):
        for i in range(NH):
            a[4 * i : 4 * i + 8 - sy, sy * NH + i] = 1.0
    a[np.arange(NH), 5 * NH + np.arange(NH)] = 1.0
    return a


def _prep_x(xb):
    """[C,H,W] fp32 -> xa phase-major fp16 [H, XCOLS].

    xa[h, BASE + v*FLAT + (r,j,c)] = x[h+v, c, 4j+r]  (0 beyond the image)."""
    t = xb.transpose(1, 2, 0)  # [h, w, c]
    pm = t.reshape(H, JP, 4, C).transpose(0, 2, 1, 3)  # [h, r, j, c]
    flat = np.ascontiguousarray(pm.reshape(H, FLAT)).astype(np.float16)
    xa = np.zeros((H, XCOLS), np.float16)
    for v in range(NV):
        xa[0 : H - v, BASE + v * FLAT : BASE + (v + 1) * FLAT] = flat[v:H]
    return xa


def _order(cells):
    """sy=0 shifts first (their stack block lands first), then by growing
    |sx| so the PE builds backlog early; (4,0) moved to the very end so
    the PE drains on a big-Lx shift instead of starving."""
    o = sorted(cells.keys(), key=lambda s: (s[0], abs(s[1])))
    o.remove((4, 0))
    o.append((4, 0))
    return o


def build_nc():
    nc = bacc.Bacc()
    xa_dram = nc.dram_tensor("xa", [H, XCOLS], fp16, kind="ExternalInput")
    amat_dram = nc.dram_tensor("amat", [H, 6 * NH], fp16, kind="ExternalInput")
    out_dram = nc.dram_tensor("out", [40, NH, NW * C], fp16,
                              kind="ExternalOutput")

    cells = _canonical_cells()
    order = _order(cells)

    with TileContext(nc) as tc:
        with (
            tc.tile_pool(name="const", bufs=1) as cpool,
            tc.tile_pool(name="q", bufs=4) as qpool,
            tc.tile_pool(name="o", bufs=3) as opool,
            tc.tile_pool(name="t4", bufs=2) as t4pool,
            tc.tile_pool(name="ps", bufs=2, space="PSUM") as ppool,
            tc.tile_pool(name="pw", bufs=1, space="PSUM") as wpool,
        ):
            amat_t = cpool.tile([H, 6 * NH], fp16)
            nc.sync.dma_start(amat_t, amat_dram[:, :])
            xa_t = cpool.tile([H, XCOLS], fp16)
            # PE warmup: keep the p-state ramped while inputs stream in
            wt = cpool.tile([H, 512], fp16)
            nc.vector.memset(wt, 0.0)
            warm_pt = wpool.tile([NH, 512], fp32)
            for _ in range(N_WARM):
                nc.tensor.matmul(warm_pt, wt[:, 0:NH], wt,
                                 start=True, stop=True)
            # chunked so the v=0 block (first consumer) lands first; v=0
            # in halves so the (0,0) square can start on the first half.
            # Issued from different engines: each engine's DGE feeds its
            # own DMA queue, so the chunks transfer in parallel instead
            # of serializing on one queue.
            bounds = [0, BASE + FLAT // 2] + \
                [BASE + v * FLAT for v in range(1, NV)] + [XCOLS]
            # v=0 halves go to two different queues so they land in
            # parallel: the (0,0) square (and everything after) starts
            # a few us sooner
            issuers = [nc.gpsimd, nc.sync, nc.gpsimd, nc.sync,
                       nc.gpsimd, nc.sync]
            for eng, (lo, hi) in zip(issuers, zip(bounds[:-1], bounds[1:])):
                eng.dma_start(xa_t[:, lo:hi], xa_dram[:, lo:hi])

            def emit_product(sy, sx, q, hv):
                s = sx % 4          # python %: s in [0,4) also for sx<0
                a = (sx - s) // 4

                def mul(flo, fhi, delta):
                    # q[h, f] = x[h, f] * x[h+sy, f+delta-sy*FLAT] on
                    # f in [flo, fhi); the sy shift is baked into the stack.
                    off = BASE + delta
                    nc.vector.tensor_mul(
                        q[0:hv, flo:fhi],
                        xa_t[0:hv, BASE + flo : BASE + fhi],
                        xa_t[0:hv, off + flo : off + fhi],
                    )

                if (sy, sx) == (0, 0):
                    # x^2 split across Act and DVE quarters so q(0,0) is
                    # ready ~as soon as the v=0 chunks land
                    qr = FLAT // 4
                    for k in range(4):
                        lo, hi = k * qr, (k + 1) * qr
                        if k % 2 == 0:
                            nc.scalar.activation(
                                q[:, lo:hi], xa_t[:, BASE + lo : BASE + hi],
                                mybir.ActivationFunctionType.Square,
                            )
                        else:
                            nc.vector.tensor_mul(
                                q[:, lo:hi],
                                xa_t[:, BASE + lo : BASE + hi],
                                xa_t[:, BASE + lo : BASE + hi],
                            )
                else:
                    lenA = (4 - s) * BLK
                    mul(0, lenA, sy * FLAT + s * BLK + C * a)
                    if s:
                        mul(lenA, FLAT,
                            sy * FLAT + (s - 4) * BLK + C * (a + 1))

            def emit_direct(sy, sx):
                hv = H - sy
                q = qpool.tile([H, FLAT], fp16, tag="q", name="qd")
                emit_product(sy, sx, q, hv)
                a_k = amat_t[0:hv, sy * NH : (sy + 1) * NH]
                xlist = list(range(max(0, -sx), 8 - max(0, sx)))
                o_t = opool.tile([NH, NW * C], fp16, tag="o", name="od")
                for ci, (n0, n1) in enumerate(N_CHUNKS):
                    pt = ppool.tile([NH, n1 - n0], fp32, tag=f"ps{ci}",
                                    name="ptd")
                    for xi, xx in enumerate(xlist):
                        base = (xx % 4) * BLK + C * (xx // 4)
                        rhs = q[0:hv, base + n0 : base + n1]
                        nc.tensor.matmul(
                            pt, a_k, rhs,
                            start=(xi == 0), stop=(xi == len(xlist) - 1),
                        )
                    nc.scalar.copy(o_t[:, n0:n1], pt)
                    # chunk DMA'd as soon as it's evacuated: the output
                    # queue drains continuously instead of piling up at
                    # the end. One write per canonical shift; the host
                    # mirrors the symmetric duplicates (out[s] == out[-s])
                    nc.gpsimd.dma_start(
                        out_dram[order.index((sy, sx))][:, n0:n1],
                        o_t[:, n0:n1])

            def emit_t4_stage1(sy):
                # width-4 phase sums for an Lx=8 (sx=0) shift: T4[i, j, c]
                # = sum_r sum_h A[h,i] Q[h, (r,j,c)], j in [0, 24)
                hv = H - sy
                q = qpool.tile([H, FLAT], fp16, tag="q", name="qt")
                emit_product(sy, 0, q, hv)
                a_k = amat_t[0:hv, sy * NH : (sy + 1) * NH]
                t4sb = t4pool.tile([NH, JP * C], fp16, tag="t4")
                for ci, n0 in enumerate((0, 512, 1024)):
                    pt = ppool.tile([NH, 512], fp32, tag=f"ps{ci}",
                                    name="ptt")
                    for ri in range(4):
                        rhs = q[0:hv, ri * BLK + n0 : ri * BLK + n0 + 512]
                        nc.tensor.matmul(pt, a_k, rhs,
                                         start=(ri == 0), stop=(ri == 3))
                    nc.scalar.copy(t4sb[:, n0 : n0 + 512], pt)
                return t4sb

            def emit_t4_finish(sy, t4sb):
                # out[i, j, c] = T4[i, j, c] + T4[i, j+1, c] via two
                # identity-matmul passes over the evacuated T4
                ident = amat_t[0:NH, 5 * NH : 6 * NH]
                o_t = opool.tile([NH, NW * C], fp16, tag="o", name="ot")
                for ci, (n0, n1) in enumerate(N_CHUNKS):
                    pt = ppool.tile([NH, n1 - n0], fp32, tag=f"ps{ci}",
                                    name="ptf")
                    nc.tensor.matmul(pt, ident, t4sb[0:NH, n0:n1],
                                     start=True, stop=False)
                    nc.tensor.matmul(pt, ident, t4sb[0:NH, C + n0 : C + n1],
                                     start=False, stop=True)
                    nc.scalar.copy(o_t[:, n0:n1], pt)
                    nc.gpsimd.dma_start(
                        out_dram[order.index((sy, 0))][:, n0:n1],
                        o_t[:, n0:n1])

            pending = None
            for (sy, sx) in order:
                if sx == 0:
                    if pending is not None:
                        emit_t4_finish(*pending)
                    pending = (sy, emit_t4_stage1(sy))
                else:
                    emit_direct(sy, sx)
                    if pending is not None:
                        emit_t4_finish(*pending)
                        pending = None
            if pending is not None:
                emit_t4_finish(*pending)

    if not nc.is_finalized():
        nc.finalize()
    return nc


@functools.lru_cache(maxsize=1)
def _get_nc():
    return build_nc()


def _in_maps(x):
    amat = _amat_np()
    return [{"xa": _prep_x(x[b]), "amat": amat} for b in range(NCORES)]


def kernel(**inputs) -> np.ndarray:
    x = np.asarray(inputs["x"], dtype=np.float32)
    assert x.shape == (B, C, H, W)
    nc = _get_nc()
    in_maps = _in_maps(x)
    res = bass_utils.run_bass_kernel_spmd(
        nc, in_maps, core_ids=list(range(NCORES)),
        trace=bool(int(os.environ.get("KERNEL_TRACE", "0"))),
    )
    outs = np.stack([r["out"] for r in res.results])  # [B, 40, i, (j c)]
    outs = outs.reshape(B, 40, NH, NW, C).astype(np.float32)
    cells = _canonical_cells()
    order = _order(cells)
    full = np.empty((B, 8, 8, NH, NW, C), np.float32)
    for ki, key in enumerate(order):
        for (dy, dx) in cells[key]:
            full[:, dy, dx] = outs[:, ki]
    # [B, dy, dx, i, j, c] -> [B, c, i, j, dy, dx]
    full = full.transpose(0, 5, 3, 4, 1, 2)
    return np.ascontiguousarray(full).astype(np.float32)


if __name__ == "__main__":
    rng = np.random.default_rng(0)
    x = rng.standard_normal((B, C, H, W), dtype=np.float32)
    y = kernel(x=x)
    print("out", y.shape, y.dtype, float(np.abs(y).max()))


# revision 34
# speedup vs baseline: 1.0162x; 1.0162x over previous
"""LocalAutoCorr2D Trainium2 kernel.

out[b,c,i,j,dy,dx] = sum_{y,x valid} x[b,c,4i+y,4j+x] * x[b,c,4i+y+sy,4j+x+sx]
with (sy,sx) = (dy-4, dx-4), windows 8x8 at stride 4 on a 96x96 image,
zero-padded at window boundaries.

Strategy (per core, batch-sharded over 8 cores):
  - out[s] == out[-s] (autocorr symmetry) -> only 40 canonical shift classes.
  - x is host-prepped into a PHASE-MAJOR fp16 layout [h, (r, j, c)] with
    w = 4j + r and c innermost, so every matmul rhs view (fixed r, a
    23-j window, all c) is one FLAT contiguous slice: the PE streams at
    full rate (a strided or multi-dim rhs runs at ~half rate). The 5
    vertical shifts v=0..4 are also host-stacked along the free dim, so
    DVE products never need cross-partition operands.
  - Per shift: product Q = x .* shift(x) on the Vector engine (fp16 2x
    mode, flat contiguous views; all shift offsets are multiples of C=64
    elements, so alignment is automatic). Vertical box-sum via 0/1-weight
    matmul (h on partitions), horizontal box-sum folded into PSUM
    accumulation across <=8 matmuls over flat rhs slices of Q.
  - Warmup matmuls bridge the PE from program start until the first
    product is ready: a PE idle gap after ramping can drop the engine
    into a sticky half-clock state for a long stretch.
  - The (0,0) square runs on the Act engine (frees the DVE, starts as
    the first input DMA chunk lands). Scalar evacuates PSUM -> SBUF as
    fp16; each canonical shift is written to DRAM once and the host
    mirrors the 24 symmetric duplicate cells.
"""

import functools
import os
import sys

import numpy as np

sys.path.insert(0, "/opt/trn_rl_repo")

import concourse.bass as bass  # noqa: E402
import concourse.bacc as bacc  # noqa: E402
import concourse.mybir as mybir  # noqa: E402
from concourse import bass_utils  # noqa: E402
from concourse.tile import TileContext  # noqa: E402

B, C, H, W = 8, 64, 96, 96
KH = KW = 8
SH = SW = 4
NH = NW = 23
NCORES = 8

JP = 24           # j' positions per r-block (w = 4j + r)
BLK = C * JP      # 1536 elements per r-block
FLAT = 4 * BLK    # 6144
NV = 5            # vertical shift copies v=0..4 stacked in the free dim
BASE = 64         # leading pad elements (AP validity for negative offsets)
TAIL = 128
XCOLS = BASE + NV * FLAT + TAIL
N_CHUNKS = [(0, 512), (512, 1024), (1024, 1472)]  # flat cols per PSUM bank
N_WARM = 42       # PE warmup: must bridge until the first product is ready
# late-order Lx=7 shifts where 2 DVE pair-sum ops (on the slack engine)
# replace 3 PE passes (on the critical engine)
D2_SHIFTS = {(3, 1), (3, -1)}

fp32 = mybir.dt.float32
fp16 = mybir.dt.float16


def _canonical_cells():
    """Map canonical shift (sy>=0, sx) -> list of output cells (dy,dx)."""
    cells = {}
    for dy in range(8):
        for dx in range(8):
            sy, sx = dy - 4, dx - 4
            key = (sy, sx) if (sy > 0 or (sy == 0 and sx >= 0)) else (-sy, -sx)
            cells.setdefault(key, []).append((dy, dx))
    assert len(cells) == 40
    return cells


def _amat_np():
    """Vertical box-sum matrices, stacked: A[h, sy*23+i] = 1 if 0<=h-4i<8-sy,
    plus a trailing 23x23 identity block (stage-2 of the T4 reduction)."""
    a = np.zeros((H, 6 * NH), np.float16)
    for sy in range(5):
        for i in range(NH):
            a[4 * i : 4 * i + 8 - sy, sy * NH + i] = 1.0
    a[np.arange(NH), 5 * NH + np.arange(NH)] = 1.0
    return a


def _prep_x(xb):
    """[C,H,W] fp32 -> xa phase-major fp16 [H, XCOLS].

    xa[h, BASE + v*FLAT + (r,j,c)] = x[h+v, c, 4j+r]  (0 beyond the image)."""
    t = xb.transpose(1, 2, 0)  # [h, w, c]
    pm = t.reshape(H, JP, 4, C).transpose(0, 2, 1, 3)  # [h, r, j, c]
    flat = np.ascontiguousarray(pm.reshape(H, FLAT)).astype(np.float16)
    xa = np.zeros((H, XCOLS), np.float16)
    for v in range(NV):
        xa[0 : H - v, BASE + v * FLAT : BASE + (v + 1) * FLAT] = flat[v:H]
    return xa


def _order(cells):
    """sy=0 shifts first (their stack block lands first), then by growing
    |sx| so the PE builds backlog early; (4,0) moved to the very end so
    the PE drains on a big-Lx shift instead of starving."""
    o = sorted(cells.keys(), key=lambda s: (s[0], abs(s[1])))
    o.remove((4, 0))
    o.append((4, 0))
    return o


def build_nc():
    nc = bacc.Bacc()
    xa_dram = nc.dram_tensor("xa", [H, XCOLS], fp16, kind="ExternalInput")
    amat_dram = nc.dram_tensor("amat", [H, 6 * NH], fp16, kind="ExternalInput")
    out_dram = nc.dram_tensor("out", [40, NH, NW * C], fp16,
                              kind="ExternalOutput")

    cells = _canonical_cells()
    order = _order(cells)

    with TileContext(nc) as tc:
        with (
            tc.tile_pool(name="const", bufs=1) as cpool,
            tc.tile_pool(name="q", bufs=4) as qpool,
            tc.tile_pool(name="o", bufs=3) as opool,
            tc.tile_pool(name="t4", bufs=2) as t4pool,
            tc.tile_pool(name="d2", bufs=2) as d2pool,
            tc.tile_pool(name="ps", bufs=2, space="PSUM") as ppool,
            tc.tile_pool(name="pw", bufs=1, space="PSUM") as wpool,
        ):
            amat_t = cpool.tile([H, 6 * NH], fp16)
            nc.sync.dma_start(amat_t, amat_dram[:, :])
            xa_t = cpool.tile([H, XCOLS], fp16)
            # PE warmup: keep the p-state ramped while inputs stream in
            wt = cpool.tile([H, 512], fp16)
            nc.vector.memset(wt, 0.0)
            warm_pt = wpool.tile([NH, 512], fp32)
            for _ in range(N_WARM):
                nc.tensor.matmul(warm_pt, wt[:, 0:NH], wt,
                                 start=True, stop=True)
            # chunked so the v=0 block (first consumer) lands first; v=0
            # in halves so the (0,0) square can start on the first half.
            # Issued from different engines: each engine's DGE feeds its
            # own DMA queue, so the chunks transfer in parallel instead
            # of serializing on one queue.
            bounds = [0, BASE + FLAT // 2] + \
                [BASE + v * FLAT for v in range(1, NV)] + [XCOLS]
            # v=0 halves go to two different queues so they land in
            # parallel: the (0,0) square (and everything after) starts
            # a few us sooner
            issuers = [nc.gpsimd, nc.sync, nc.gpsimd, nc.sync,
                       nc.gpsimd, nc.sync]
            for eng, (lo, hi) in zip(issuers, zip(bounds[:-1], bounds[1:])):
                eng.dma_start(xa_t[:, lo:hi], xa_dram[:, lo:hi])

            def emit_product(sy, sx, q, hv):
                s = sx % 4          # python %: s in [0,4) also for sx<0
                a = (sx - s) // 4

                def mul(flo, fhi, delta):
                    # q[h, f] = x[h, f] * x[h+sy, f+delta-sy*FLAT] on
                    # f in [flo, fhi); the sy shift is baked into the stack.
                    off = BASE + delta
                    nc.vector.tensor_mul(
                        q[0:hv, flo:fhi],
                        xa_t[0:hv, BASE + flo : BASE + fhi],
                        xa_t[0:hv, off + flo : off + fhi],
                    )

                if (sy, sx) == (0, 0):
                    # x^2 split across Act and DVE quarters so q(0,0) is
                    # ready ~as soon as the v=0 chunks land
                    qr = FLAT // 4
                    for k in range(4):
                        lo, hi = k * qr, (k + 1) * qr
                        if k % 2 == 0:
                            nc.scalar.activation(
                                q[:, lo:hi], xa_t[:, BASE + lo : BASE + hi],
                                mybir.ActivationFunctionType.Square,
                            )
                        else:
                            nc.vector.tensor_mul(
                                q[:, lo:hi],
                                xa_t[:, BASE + lo : BASE + hi],
                                xa_t[:, BASE + lo : BASE + hi],
                            )
                else:
                    lenA = (4 - s) * BLK
                    mul(0, lenA, sy * FLAT + s * BLK + C * a)
                    if s:
                        mul(lenA, FLAT,
                            sy * FLAT + (s - 4) * BLK + C * (a + 1))

            def emit_direct(sy, sx):
                hv = H - sy
                q = qpool.tile([H, FLAT], fp16, tag="q", name="qd")
                emit_product(sy, sx, q, hv)
                a_k = amat_t[0:hv, sy * NH : (sy + 1) * NH]
                x0 = max(0, -sx)
                xlist = list(range(x0, 8 - max(0, sx)))
                use_d2 = (sy, sx) in D2_SHIFTS
                if use_d2:
                    # d2[slot] = q[r-block k] + q[next w-phase]: each pair
                    # of adjacent x-offsets becomes a single PE pass
                    P = len(xlist) // 2
                    d2 = d2pool.tile([H, 2 * BLK], fp16, tag="d2")
                    slots = {}
                    for t in range(P):
                        k = (x0 + 2 * t) % 4
                        if k in slots:
                            continue
                        slots[k] = len(slots)
                        lo = slots[k] * BLK
                        in2 = q[0:hv, (k + 1) * BLK : (k + 2) * BLK] \
                            if k < 3 else q[0:hv, C : C + BLK]
                        nc.vector.tensor_add(
                            d2[0:hv, lo : lo + BLK],
                            q[0:hv, k * BLK : (k + 1) * BLK], in2)
                o_t = opool.tile([NH, NW * C], fp16, tag="o", name="od")
                for ci, (n0, n1) in enumerate(N_CHUNKS):
                    pt = ppool.tile([NH, n1 - n0], fp32, tag=f"ps{ci}",
                                    name="ptd")
                    if use_d2:
                        npass = P + (len(xlist) & 1)
                        for t in range(P):
                            xx = x0 + 2 * t
                            base = slots[xx % 4] * BLK + C * (xx // 4)
                            nc.tensor.matmul(
                                pt, a_k, d2[0:hv, base + n0 : base + n1],
                                start=(t == 0), stop=(t == npass - 1),
                            )
                        if len(xlist) & 1:
                            xx = xlist[-1]
                            base = (xx % 4) * BLK + C * (xx // 4)
                            nc.tensor.matmul(
                                pt, a_k, q[0:hv, base + n0 : base + n1],
                                start=False, stop=True,
                            )
                    else:
                        for xi, xx in enumerate(xlist):
                            base = (xx % 4) * BLK + C * (xx // 4)
                            rhs = q[0:hv, base + n0 : base + n1]
                            nc.tensor.matmul(
                                pt, a_k, rhs,
                                start=(xi == 0), stop=(xi == len(xlist) - 1),
                            )
                    nc.scalar.copy(o_t[:, n0:n1], pt)
                    # chunk DMA'd as soon as it's evacuated: the output
                    # queue drains continuously instead of piling up at
                    # the end. One write per canonical shift; the host
                    # mirrors the symmetric duplicates (out[s] == out[-s])
                    nc.gpsimd.dma_start(
                        out_dram[order.index((sy, sx))][:, n0:n1],
                        o_t[:, n0:n1])

            def emit_t4_stage1(sy):
                # width-4 phase sums for an Lx=8 (sx=0) shift: T4[i, j, c]
                # = sum_r sum_h A[h,i] Q[h, (r,j,c)], j in [0, 24)
                hv = H - sy
                q = qpool.tile([H, FLAT], fp16, tag="q", name="qt")
                emit_product(sy, 0, q, hv)
                a_k = amat_t[0:hv, sy * NH : (sy + 1) * NH]
                t4sb = t4pool.tile([NH, JP * C], fp16, tag="t4")
                for ci, n0 in enumerate((0, 512, 1024)):
                    pt = ppool.tile([NH, 512], fp32, tag=f"ps{ci}",
                                    name="ptt")
                    for ri in range(4):
                        rhs = q[0:hv, ri * BLK + n0 : ri * BLK + n0 + 512]
                        nc.tensor.matmul(pt, a_k, rhs,
                                         start=(ri == 0), stop=(ri == 3))
                    nc.scalar.copy(t4sb[:, n0 : n0 + 512], pt)
                return t4sb

            def emit_t4_finish(sy, t4sb):
                # out[i, j, c] = T4[i, j, c] + T4[i, j+1, c] via two
                # identity-matmul passes over the evacuated T4
                ident = amat_t[0:NH, 5 * NH : 6 * NH]
                o_t = opool.tile([NH, NW * C], fp16, tag="o", name="ot")
                for ci, (n0, n1) in enumerate(N_CHUNKS):
                    pt = ppool.tile([NH, n1 - n0], fp32, tag=f"ps{ci}",
                                    name="ptf")
                    nc.tensor.matmul(pt, ident, t4sb[0:NH, n0:n1],
                                     start=True, stop=False)
                    nc.tensor.matmul(pt, ident, t4sb[0:NH, C + n0 : C + n1],
                                     start=False, stop=True)
                    nc.scalar.copy(o_t[:, n0:n1], pt)
                    nc.gpsimd.dma_start(
                        out_dram[order.index((sy, 0))][:, n0:n1],
                        o_t[:, n0:n1])

            pending = None
            for (sy, sx) in order:
                if sx == 0:
                    if pending is not None:
                        emit_t4_finish(*pending)
                    pending = (sy, emit_t4_stage1(sy))
                else:
                    emit_direct(sy, sx)
                    if pending is not None:
                        emit_t4_finish(*pending)
                        pending = None
            if pending is not None:
                emit_t4_finish(*pending)

    if not nc.is_finalized():
        nc.finalize()
    return nc


@functools.lru_cache(maxsize=1)
def _get_nc():
    return build_nc()


def _in_maps(x):
    amat = _amat_np()
    return [{"xa": _prep_x(x[b]), "amat": amat} for b in range(NCORES)]


def kernel(**inputs) -> np.ndarray:
    x = np.asarray(inputs["x"], dtype=np.float32)
    assert x.shape == (B, C, H, W)
    nc = _get_nc()
    in_maps = _in_maps(x)
    res = bass_utils.run_bass_kernel_spmd(
        nc, in_maps, core_ids=list(range(NCORES)),
        trace=bool(int(os.environ.get("KERNEL_TRACE", "0"))),
    )
    outs = np.stack([r["out"] for r in res.results])  # [B, 40, i, (j c)]
    outs = outs.reshape(B, 40, NH, NW, C).astype(np.float32)
    cells = _canonical_cells()
    order = _order(cells)
    full = np.empty((B, 8, 8, NH, NW, C), np.float32)
    for ki, key in enumerate(order):
        for (dy, dx) in cells[key]:
            full[:, dy, dx] = outs[:, ki]
    # [B, dy, dx, i, j, c] -> [B, c, i, j, dy, dx]
    full = full.transpose(0, 5, 3, 4, 1, 2)
    return np.ascontiguousarray(full).astype(np.float32)


if __name__ == "__main__":
    rng = np.random.default_rng(0)
    x = rng.standard_normal((B, C, H, W), dtype=np.float32)
    y = kernel(x=x)
    print("out", y.shape, y.dtype, float(np.abs(y).max()))


# revision 35
# speedup vs baseline: 1.0275x; 1.0111x over previous
"""LocalAutoCorr2D Trainium2 kernel.

out[b,c,i,j,dy,dx] = sum_{y,x valid} x[b,c,4i+y,4j+x] * x[b,c,4i+y+sy,4j+x+sx]
with (sy,sx) = (dy-4, dx-4), windows 8x8 at stride 4 on a 96x96 image,
zero-padded at window boundaries.

Strategy (per core, batch-sharded over 8 cores):
  - out[s] == out[-s] (autocorr symmetry) -> only 40 canonical shift classes.
  - x is host-prepped into a PHASE-MAJOR fp16 layout [h, (r, j, c)] with
    w = 4j + r and c innermost, so every matmul rhs view (fixed r, a
    23-j window, all c) is one FLAT contiguous slice: the PE streams at
    full rate (a strided or multi-dim rhs runs at ~half rate). The 5
    vertical shifts v=0..4 are also host-stacked along the free dim, so
    DVE products never need cross-partition operands.
  - Per shift: product Q = x .* shift(x) on the Vector engine (fp16 2x
    mode, flat contiguous views; all shift offsets are multiples of C=64
    elements, so alignment is automatic). Vertical box-sum via 0/1-weight
    matmul (h on partitions), horizontal box-sum folded into PSUM
    accumulation across <=8 matmuls over flat rhs slices of Q.
  - Warmup matmuls bridge the PE from program start until the first
    product is ready: a PE idle gap after ramping can drop the engine
    into a sticky half-clock state for a long stretch.
  - The (0,0) square runs on the Act engine (frees the DVE, starts as
    the first input DMA chunk lands). Scalar evacuates PSUM -> SBUF as
    fp16; each canonical shift is written to DRAM once and the host
    mirrors the 24 symmetric duplicate cells.
"""

import functools
import os
import sys

import numpy as np

sys.path.insert(0, "/opt/trn_rl_repo")

import concourse.bass as bass  # noqa: E402
import concourse.bacc as bacc  # noqa: E402
import concourse.mybir as mybir  # noqa: E402
from concourse import bass_utils  # noqa: E402
from concourse.tile import TileContext  # noqa: E402

B, C, H, W = 8, 64, 96, 96
KH = KW = 8
SH = SW = 4
NH = NW = 23
NCORES = 8

JP = 24           # j' positions per r-block (w = 4j + r)
BLK = C * JP      # 1536 elements per r-block
FLAT = 4 * BLK    # 6144
NV = 5            # vertical shift copies v=0..4 stacked in the free dim
BASE = 64         # leading pad elements (AP validity for negative offsets)
TAIL = 128
XCOLS = BASE + NV * FLAT + TAIL
N_CHUNKS = [(0, 512), (512, 1024), (1024, 1472)]  # flat cols per PSUM bank
N_WARM = 42       # PE warmup: must bridge until the first product is ready
# late-order Lx=7 shifts where 2 DVE pair-sum ops (on the slack engine)
# replace 3 PE passes (on the critical engine)
D2_SHIFTS = {(3, 1), (3, -1)}

fp32 = mybir.dt.float32
fp16 = mybir.dt.float16


def _canonical_cells():
    """Map canonical shift (sy>=0, sx) -> list of output cells (dy,dx)."""
    cells = {}
    for dy in range(8):
        for dx in range(8):
            sy, sx = dy - 4, dx - 4
            key = (sy, sx) if (sy > 0 or (sy == 0 and sx >= 0)) else (-sy, -sx)
            cells.setdefault(key, []).append((dy, dx))
    assert len(cells) == 40
    return cells


def _amat_np():
    """Vertical box-sum matrices, stacked: A[h, sy*23+i] = 1 if 0<=h-4i<8-sy,
    plus a trailing 23x23 identity block (stage-2 of the T4 reduction)."""
    a = np.zeros((H, 6 * NH), np.float16)
    for sy in range(5):
        for i in range(NH):
            a[4 * i : 4 * i + 8 - sy, sy * NH + i] = 1.0
    a[np.arange(NH), 5 * NH + np.arange(NH)] = 1.0
    return a


def _prep_x(xb):
    """[C,H,W] fp32 -> xa phase-major fp16 [H, XCOLS].

    xa[h, BASE + v*FLAT + (r,j,c)] = x[h+v, c, 4j+r]  (0 beyond the image)."""
    t = xb.transpose(1, 2, 0)  # [h, w, c]
    pm = t.reshape(H, JP, 4, C).transpose(0, 2, 1, 3)  # [h, r, j, c]
    flat = np.ascontiguousarray(pm.reshape(H, FLAT)).astype(np.float16)
    xa = np.zeros((H, XCOLS), np.float16)
    for v in range(NV):
        xa[0 : H - v, BASE + v * FLAT : BASE + (v + 1) * FLAT] = flat[v:H]
    return xa


def _order(cells):
    """sy=0 shifts first (their stack block lands first), then by growing
    |sx| so the PE builds backlog early; (4,0) moved to the very end so
    the PE drains on a big-Lx shift instead of starving."""
    o = sorted(cells.keys(), key=lambda s: (s[0], abs(s[1])))
    o.remove((4, 0))
    o.append((4, 0))
    return o


def build_nc():
    nc = bacc.Bacc()
    xa_dram = nc.dram_tensor("xa", [H, XCOLS], fp16, kind="ExternalInput")
    amat_dram = nc.dram_tensor("amat", [H, 6 * NH], fp16, kind="ExternalInput")
    out_dram = nc.dram_tensor("out", [40, NH, NW * C], fp16,
                              kind="ExternalOutput")

    cells = _canonical_cells()
    order = _order(cells)

    with TileContext(nc) as tc:
        with (
            tc.tile_pool(name="const", bufs=1) as cpool,
            tc.tile_pool(name="q", bufs=6) as qpool,
            tc.tile_pool(name="o", bufs=4) as opool,
            tc.tile_pool(name="t4", bufs=2) as t4pool,
            tc.tile_pool(name="d2", bufs=2) as d2pool,
            tc.tile_pool(name="ps", bufs=2, space="PSUM") as ppool,
            tc.tile_pool(name="pw", bufs=1, space="PSUM") as wpool,
        ):
            amat_t = cpool.tile([H, 6 * NH], fp16)
            nc.sync.dma_start(amat_t, amat_dram[:, :])
            xa_t = cpool.tile([H, XCOLS], fp16)
            # PE warmup: keep the p-state ramped while inputs stream in
            wt = cpool.tile([H, 512], fp16)
            nc.vector.memset(wt, 0.0)
            warm_pt = wpool.tile([NH, 512], fp32)
            for _ in range(N_WARM):
                nc.tensor.matmul(warm_pt, wt[:, 0:NH], wt,
                                 start=True, stop=True)
            # chunked so the v=0 block (first consumer) lands first; v=0
            # in halves so the (0,0) square can start on the first half.
            # Issued from different engines: each engine's DGE feeds its
            # own DMA queue, so the chunks transfer in parallel instead
            # of serializing on one queue.
            bounds = [0, BASE + FLAT // 2] + \
                [BASE + v * FLAT for v in range(1, NV)] + [XCOLS]
            # v=0 halves go to two different queues so they land in
            # parallel: the (0,0) square (and everything after) starts
            # a few us sooner
            issuers = [nc.gpsimd, nc.sync, nc.gpsimd, nc.sync,
                       nc.gpsimd, nc.sync]
            for eng, (lo, hi) in zip(issuers, zip(bounds[:-1], bounds[1:])):
                eng.dma_start(xa_t[:, lo:hi], xa_dram[:, lo:hi])

            def emit_product(sy, sx, q, hv):
                s = sx % 4          # python %: s in [0,4) also for sx<0
                a = (sx - s) // 4

                def mul(flo, fhi, delta):
                    # q[h, f] = x[h, f] * x[h+sy, f+delta-sy*FLAT] on
                    # f in [flo, fhi); the sy shift is baked into the stack.
                    off = BASE + delta
                    nc.vector.tensor_mul(
                        q[0:hv, flo:fhi],
                        xa_t[0:hv, BASE + flo : BASE + fhi],
                        xa_t[0:hv, off + flo : off + fhi],
                    )

                if (sy, sx) == (0, 0):
                    # x^2 split across Act and DVE quarters so q(0,0) is
                    # ready ~as soon as the v=0 chunks land
                    qr = FLAT // 4
                    for k in range(4):
                        lo, hi = k * qr, (k + 1) * qr
                        if k % 2 == 0:
                            nc.scalar.activation(
                                q[:, lo:hi], xa_t[:, BASE + lo : BASE + hi],
                                mybir.ActivationFunctionType.Square,
                            )
                        else:
                            nc.vector.tensor_mul(
                                q[:, lo:hi],
                                xa_t[:, BASE + lo : BASE + hi],
                                xa_t[:, BASE + lo : BASE + hi],
                            )
                else:
                    lenA = (4 - s) * BLK
                    mul(0, lenA, sy * FLAT + s * BLK + C * a)
                    if s:
                        mul(lenA, FLAT,
                            sy * FLAT + (s - 4) * BLK + C * (a + 1))

            def emit_direct(sy, sx):
                hv = H - sy
                q = qpool.tile([H, FLAT], fp16, tag="q", name="qd")
                emit_product(sy, sx, q, hv)
                a_k = amat_t[0:hv, sy * NH : (sy + 1) * NH]
                x0 = max(0, -sx)
                xlist = list(range(x0, 8 - max(0, sx)))
                use_d2 = (sy, sx) in D2_SHIFTS
                if use_d2:
                    # d2[slot] = q[r-block k] + q[next w-phase]: each pair
                    # of adjacent x-offsets becomes a single PE pass
                    P = len(xlist) // 2
                    d2 = d2pool.tile([H, 2 * BLK], fp16, tag="d2")
                    slots = {}
                    for t in range(P):
                        k = (x0 + 2 * t) % 4
                        if k in slots:
                            continue
                        slots[k] = len(slots)
                        lo = slots[k] * BLK
                        in2 = q[0:hv, (k + 1) * BLK : (k + 2) * BLK] \
                            if k < 3 else q[0:hv, C : C + BLK]
                        nc.vector.tensor_add(
                            d2[0:hv, lo : lo + BLK],
                            q[0:hv, k * BLK : (k + 1) * BLK], in2)
                o_t = opool.tile([NH, NW * C], fp16, tag="o", name="od")
                for ci, (n0, n1) in enumerate(N_CHUNKS):
                    pt = ppool.tile([NH, n1 - n0], fp32, tag=f"ps{ci}",
                                    name="ptd")
                    if use_d2:
                        npass = P + (len(xlist) & 1)
                        for t in range(P):
                            xx = x0 + 2 * t
                            base = slots[xx % 4] * BLK + C * (xx // 4)
                            nc.tensor.matmul(
                                pt, a_k, d2[0:hv, base + n0 : base + n1],
                                start=(t == 0), stop=(t == npass - 1),
                            )
                        if len(xlist) & 1:
                            xx = xlist[-1]
                            base = (xx % 4) * BLK + C * (xx // 4)
                            nc.tensor.matmul(
                                pt, a_k, q[0:hv, base + n0 : base + n1],
                                start=False, stop=True,
                            )
                    else:
                        for xi, xx in enumerate(xlist):
                            base = (xx % 4) * BLK + C * (xx // 4)
                            rhs = q[0:hv, base + n0 : base + n1]
                            nc.tensor.matmul(
                                pt, a_k, rhs,
                                start=(xi == 0), stop=(xi == len(xlist) - 1),
                            )
                    nc.scalar.copy(o_t[:, n0:n1], pt)
                    # chunk DMA'd as soon as it's evacuated: the output
                    # queue drains continuously instead of piling up at
                    # the end. One write per canonical shift; the host
                    # mirrors the symmetric duplicates (out[s] == out[-s])
                    nc.gpsimd.dma_start(
                        out_dram[order.index((sy, sx))][:, n0:n1],
                        o_t[:, n0:n1])

            def emit_t4_stage1(sy):
                # width-4 phase sums for an Lx=8 (sx=0) shift: T4[i, j, c]
                # = sum_r sum_h A[h,i] Q[h, (r,j,c)], j in [0, 24)
                hv = H - sy
                q = qpool.tile([H, FLAT], fp16, tag="q", name="qt")
                emit_product(sy, 0, q, hv)
                a_k = amat_t[0:hv, sy * NH : (sy + 1) * NH]
                t4sb = t4pool.tile([NH, JP * C], fp16, tag="t4")
                for ci, n0 in enumerate((0, 512, 1024)):
                    pt = ppool.tile([NH, 512], fp32, tag=f"ps{ci}",
                                    name="ptt")
                    for ri in range(4):
                        rhs = q[0:hv, ri * BLK + n0 : ri * BLK + n0 + 512]
                        nc.tensor.matmul(pt, a_k, rhs,
                                         start=(ri == 0), stop=(ri == 3))
                    nc.scalar.copy(t4sb[:, n0 : n0 + 512], pt)
                return t4sb

            def emit_t4_finish(sy, t4sb):
                # out[i, j, c] = T4[i, j, c] + T4[i, j+1, c] via two
                # identity-matmul passes over the evacuated T4
                ident = amat_t[0:NH, 5 * NH : 6 * NH]
                o_t = opool.tile([NH, NW * C], fp16, tag="o", name="ot")
                for ci, (n0, n1) in enumerate(N_CHUNKS):
                    pt = ppool.tile([NH, n1 - n0], fp32, tag=f"ps{ci}",
                                    name="ptf")
                    nc.tensor.matmul(pt, ident, t4sb[0:NH, n0:n1],
                                     start=True, stop=False)
                    nc.tensor.matmul(pt, ident, t4sb[0:NH, C + n0 : C + n1],
                                     start=False, stop=True)
                    nc.scalar.copy(o_t[:, n0:n1], pt)
                    nc.gpsimd.dma_start(
                        out_dram[order.index((sy, 0))][:, n0:n1],
                        o_t[:, n0:n1])

            pending = None
            for (sy, sx) in order:
                if sx == 0:
                    if pending is not None:
                        emit_t4_finish(*pending)
                    pending = (sy, emit_t4_stage1(sy))
                else:
                    emit_direct(sy, sx)
                    if pending is not None:
                        emit_t4_finish(*pending)
                        pending = None
            if pending is not None:
                emit_t4_finish(*pending)

    if not nc.is_finalized():
        nc.finalize()
    return nc


@functools.lru_cache(maxsize=1)
def _get_nc():
    return build_nc()


def _in_maps(x):
    amat = _amat_np()
    return [{"xa": _prep_x(x[b]), "amat": amat} for b in range(NCORES)]


def kernel(**inputs) -> np.ndarray:
    x = np.asarray(inputs["x"], dtype=np.float32)
    assert x.shape == (B, C, H, W)
    nc = _get_nc()
    in_maps = _in_maps(x)
    res = bass_utils.run_bass_kernel_spmd(
        nc, in_maps, core_ids=list(range(NCORES)),
        trace=bool(int(os.environ.get("KERNEL_TRACE", "0"))),
    )
    outs = np.stack([r["out"] for r in res.results])  # [B, 40, i, (j c)]
    outs = outs.reshape(B, 40, NH, NW, C).astype(np.float32)
    cells = _canonical_cells()
    order = _order(cells)
    full = np.empty((B, 8, 8, NH, NW, C), np.float32)
    for ki, key in enumerate(order):
        for (dy, dx) in cells[key]:
            full[:, dy, dx] = outs[:, ki]
    # [B, dy, dx, i, j, c] -> [B, c, i, j, dy, dx]
    full = full.transpose(0, 5, 3, 4, 1, 2)
    return np.ascontiguousarray(full).astype(np.float32)


if __name__ == "__main__":
    rng = np.random.default_rng(0)
    x = rng.standard_normal((B, C, H, W), dtype=np.float32)
    y = kernel(x=x)
    print("out", y.shape, y.dtype, float(np.abs(y).max()))
